# revision 1
# baseline (speedup 1.0000x reference)
"""Causal single-head attention (B=4, N=4096, d_in=1024, d_inner=512, d_out=1024)
for 8 Trainium2 NeuronCores.

Sharding: core c -> (batch b=c//2, half h=c%2). Each core handles the 4
global 512-row query blocks {2u+h : u=0..3} of batch b (block-interleaved
for causal load balance) and computes the full K/V projection on-chip.
No collectives; causality enters only through a per-core 0/1 mask input,
so the instruction stream is SPMD-uniform.

All matmuls run in float32r (full PE rate at free-dim>=256, ~1.5e-4 rel
err); the attention-probability @ V stage runs in bf16. Layouts are chosen
fully transposed (scores [j,i], attention output [dv,i]) so the kernel
contains no transposes at all.
"""

import sys

if "/opt/trn_rl_repo" not in sys.path:
    sys.path.insert(0, "/opt/trn_rl_repo")

import numpy as np

import concourse.bacc as bacc
import concourse.mybir as mybir
import concourse.tile as tile
from concourse.bass_utils import run_bass_kernel_spmd

P = 128
B, N, DIN, DI, DO = 4, 4096, 1024, 512, 1024
NCORES = 8
NQ = N // 2          # query rows per core (2048)
NU = 4               # query super-blocks of 512 per core
NJS = N // 512       # key strips of 512 (8)
NKB = N // P         # key blocks of 128 (32)
SCALE = float(DI) ** -0.5

F32 = mybir.dt.float32
F32R = mybir.dt.float32r
BF16 = mybir.dt.bfloat16
FP16 = mybir.dt.float16
AF = mybir.ActivationFunctionType

_COMPILED = None


def _build():
    nc = bacc.Bacc(None, target_bir_lowering=False)

    xt_d = nc.dram_tensor("xt", [DIN, N], F32R, kind="ExternalInput")
    xqt_d = nc.dram_tensor("xqt", [DIN, NQ], F32R, kind="ExternalInput")
    wq_d = nc.dram_tensor("wq", [DIN, DI], F32R, kind="ExternalInput")
    wk_d = nc.dram_tensor("wk", [DIN, DI], F32R, kind="ExternalInput")
    wv_d = nc.dram_tensor("wv", [DIN, DI], F32R, kind="ExternalInput")
    wout_d = nc.dram_tensor("wout", [DI, DO], F32R, kind="ExternalInput")
    bout_d = nc.dram_tensor("bout", [P, DO], F32R, kind="ExternalInput")
    mask_d = nc.dram_tensor("mask", [P, 8, 512], FP16, kind="ExternalInput")
    y_d = nc.dram_tensor("y", [NQ, DO], F32, kind="ExternalOutput")

    with tile.TileContext(nc) as tc:
        with tc.tile_pool(name="persist", bufs=1) as pp:
            kt = pp.tile([P, 4, N], F32R)        # K^T  [dk, j]   64KB/part
            vt = pp.tile([P, NKB, DI], FP16)     # V    [j, dv]   16KB/part
            wq = pp.tile([P, 8, DI], F32R)
            bout = pp.tile([P, DO], F32R)
            ones = pp.tile([P, P], FP16)
            ones_r = pp.tile([P, P], F32R)
            xqs_t = [pp.tile([P, 8, 512], F32R, name=f"xq{u}", tag="xqs", bufs=1)
                     for u in range(NU)]

            def qproj(u, qt):
                """Q^T projection for super-block u into qt [P, 4, 512]."""
                for dq in range(4):
                    qps = psB.tile([P, 512], F32, name=f"q{u}_{dq}", tag="ka", bufs=3)
                    for c in range(8):
                        nc.tensor.matmul(
                            qps[:], wq[:, c, dq * P:(dq + 1) * P], xqs_t[u][:, c, :],
                            start=(c == 0), stop=(c == 7),
                        )
                    nc.vector.tensor_copy(qt[:, dq, :], qps[:])

            # ---- Phase A: K^T and V projection over all N keys ----
            # DMA order: phase-A needs first, phase-B inputs trickled between.
            pa = tc.tile_pool(name="phaseA", bufs=1)
            wp = pa.__enter__()
            psKV_cm = tc.tile_pool(name="psKV", bufs=1, space="PSUM")
            psKV = psKV_cm.__enter__()
            wk = wp.tile([P, 8, DI], F32R)
            wv = wp.tile([P, 8, DI], F32R)
            wk_src = wk_d.ap().rearrange("(a p) n -> p a n", p=P)
            for c in range(8):
                nc.gpsimd.dma_start(wk[:, c, :], wk_src[:, c, :])
            wv_src = wv_d.ap().rearrange("(a p) n -> p a n", p=P)
            for c in range(8):
                nc.gpsimd.dma_start(wv[:, c, :], wv_src[:, c, :])
            late_dmas = {
                2: lambda: nc.sync.dma_start(
                    xqs_t[0][:],
                    xqt_d.ap()[:, 0:512].rearrange("(a p) j -> p a j", p=P)),
                4: lambda: nc.sync.dma_start(
                    wq[:], wq_d.ap().rearrange("(a p) n -> p a n", p=P)),
                6: lambda: nc.sync.dma_start(bout[:], bout_d.ap()),
            }
            nc.vector.memset(ones[:], 1.0)
            nc.vector.tensor_copy(ones_r[:], ones[:])
            for js in range(NJS):
                xs = wp.tile([P, 8, 512], F32R, name=f"xs{js}", tag="xs", bufs=2)
                xs_src = xt_d.ap()[:, js * 512:(js + 1) * 512].rearrange(
                    "(a p) j -> p a j", p=P
                )
                if js == 0:
                    for c in range(8):
                        nc.sync.dma_start(xs[:, c, :], xs_src[:, c, :])
                else:
                    nc.sync.dma_start(xs[:, 0:4, :], xs_src[:, 0:4, :])
                    nc.gpsimd.dma_start(xs[:, 4:8, :], xs_src[:, 4:8, :])
                if js in late_dmas:
                    late_dmas[js]()
                for dk in range(4):
                    kps = psKV.tile([P, 512], F32, name=f"k{js}_{dk}", tag="kv", bufs=8)
                    for c in range(8):
                        nc.tensor.matmul(
                            kps[:], wk[:, c, dk * P:(dk + 1) * P], xs[:, c, :],
                            start=(c == 0), stop=(c == 7),
                        )
                    nc.vector.tensor_copy(kt[:, dk, js * 512:(js + 1) * 512], kps[:])
                for jsub in range(4):
                    vps = psKV.tile([P, 512], F32, name=f"v{js}_{jsub}", tag="kv", bufs=8)
                    for c in range(8):
                        nc.tensor.matmul(
                            vps[:], xs[:, c, jsub * P:(jsub + 1) * P], wv[:, c, :],
                            start=(c == 0), stop=(c == 7),
                        )
                    nc.vector.tensor_copy(vt[:, js * 4 + jsub, :], vps[:])

            psKV_cm.__exit__(None, None, None)
            pa.__exit__(None, None, None)

            # ---- Phase B: per query super-block u ----
            pb = tc.tile_pool(name="phaseB", bufs=1)
            wp = pb.__enter__()
            psB_cm = tc.tile_pool(name="psB", bufs=1, space="PSUM")
            psB = psB_cm.__enter__()
            mask = wp.tile([P, 8, 512], FP16)
            wout = wp.tile([P, 4, DO], F32R)
            nc.sync.dma_start(mask[:], mask_d.ap())
            nc.sync.dma_start(wout[:], wout_d.ap().rearrange("(a p) n -> p a n", p=P))

            qts = [wp.tile([P, 4, 512], F32R, name=f"qt{u}", tag="qt", bufs=2)
                   for u in range(NU)]
            qproj(0, qts[0])  # runs right after phase A matmuls

            LAG = 2
            for u in range(NU):
                nkb = 8 * u + 8  # key blocks (128) this super-block attends to
                qt = qts[u]
                if u + 1 < NU:
                    nc.gpsimd.dma_start(
                        xqs_t[u + 1][:],
                        xqt_d.ap()[:, (u + 1) * 512:(u + 2) * 512].rearrange(
                            "(a p) j -> p a j", p=P
                        ),
                    )

                outT = [
                    psB.tile([P, 512], F32, name=f"o{u}_{d}", tag=f"outT{d}", bufs=1)
                    for d in range(4)
                ]
                l_ps = psB.tile([P, 512], F32, name=f"l{u}", tag="l", bufs=1)
                p_ts = []

                def attn_v(kb):
                    pt = p_ts[kb]
                    for dvc in range(4):
                        nc.tensor.matmul(
                            outT[dvc][:], vt[:, kb, dvc * P:(dvc + 1) * P], pt[:],
                            start=(kb == 0), stop=(kb == nkb - 1),
                        )
                    nc.tensor.matmul(
                        l_ps[:], ones[:], pt[:],
                        start=(kb == 0), stop=(kb == nkb - 1),
                    )

                for kb in range(nkb):
                    s_ps = psB.tile(
                        [P, 512], F32, name=f"s{u}_{kb}", tag="ka", bufs=3
                    )
                    for dkc in range(4):
                        nc.tensor.matmul(
                            s_ps[:], kt[:, dkc, kb * P:(kb + 1) * P], qt[:, dkc, :],
                            start=(dkc == 0), stop=(dkc == 3),
                        )
                    pt = wp.tile([P, 512], FP16, name=f"p{u}_{kb}", tag="pt", bufs=4)
                    p_ts.append(pt)
                    kb_l = kb - (nkb - 8)
                    if kb_l >= 0:
                        # halves: the DVE mask-mul on half 0 runs under the
                        # ScalarE exp of half 1
                        nc.scalar.activation(pt[:, 0:256], s_ps[:, 0:256],
                                             AF.Exp, scale=SCALE)
                        nc.scalar.activation(pt[:, 256:512], s_ps[:, 256:512],
                                             AF.Exp, scale=SCALE)
                        nc.vector.tensor_mul(pt[:, 0:256], pt[:, 0:256],
                                             mask[:, kb_l, 0:256])
                        nc.vector.tensor_mul(pt[:, 256:512], pt[:, 256:512],
                                             mask[:, kb_l, 256:512])
                    else:
                        nc.scalar.activation(pt[:], s_ps[:], AF.Exp, scale=SCALE)
                    if kb >= LAG:
                        attn_v(kb - LAG)
                for kb in range(nkb - LAG, nkb):
                    attn_v(kb)

                # PE: Q projection of u+1 covers the DVE normalization chain;
                # its PSUM evictions go first in the DVE queue so the
                # out-projection's PSUM slots free early.
                if u + 1 < NU:
                    qproj(u + 1, qts[u + 1])

                recip = wp.tile([P, 512], F32, name=f"r{u}", tag="recip", bufs=1)
                nc.vector.reciprocal(recip[:], l_ps[:])
                attn = [
                    wp.tile([P, 512], F32R, name=f"a{u}_{d}", tag=f"attn{d}", bufs=1)
                    for d in range(4)
                ]
                for dvc in range(4):
                    nc.vector.tensor_mul(attn[dvc][:], outT[dvc][:], recip[:])

                for ic in range(4):
                    y_s = wp.tile([P, DO], F32, name=f"y{u}_{ic}", tag="ys", bufs=2)
                    for doc in range(2):
                        y_ps = psB.tile(
                            [P, 512], F32, name=f"yp{u}_{ic}_{doc}", tag="ka", bufs=3
                        )
                        nc.tensor.matmul(
                            y_ps[:], ones_r[0:1, :],
                            bout[0:1, doc * 512:(doc + 1) * 512],
                            start=True, stop=False,
                        )
                        for dvc in range(4):
                            nc.tensor.matmul(
                                y_ps[:],
                                attn[dvc][:, ic * P:(ic + 1) * P],
                                wout[:, dvc, doc * 512:(doc + 1) * 512],
                                start=False, stop=(dvc == 3),
                            )
                        nc.scalar.activation(
                            y_s[:, doc * 512:(doc + 1) * 512], y_ps[:], AF.Copy
                        )
                    nc.sync.dma_start(
                        y_d.ap().rearrange("(a p) n -> p a n", p=P)[:, u * 4 + ic, :],
                        y_s[:],
                    )
            psB_cm.__exit__(None, None, None)
            pb.__exit__(None, None, None)

    nc.compile()
    return nc


def _get_nc():
    global _COMPILED
    if _COMPILED is None:
        _COMPILED = _build()
    return _COMPILED


def _make_mask(h: int) -> np.ndarray:
    # mask[p_j, kb_l, i_l] = 1 if key (kb_l*128 + p_j) - i_l <= h*512 else 0
    pj = np.arange(P)[:, None, None]
    kb_l = np.arange(8)[None, :, None]
    il = np.arange(512)[None, None, :]
    return ((kb_l * P + pj - il) <= h * 512).astype(np.float16)


def _prep_inputs(x, w_qkv, w_out, b_out):
    wq = np.ascontiguousarray(w_qkv[:, 0:DI])
    wk = np.ascontiguousarray(w_qkv[:, DI:2 * DI])
    wv = np.ascontiguousarray(w_qkv[:, 2 * DI:3 * DI])
    bout = np.broadcast_to(b_out.astype(np.float32), (P, DO)).copy()
    masks = [_make_mask(h) for h in range(2)]
    in_maps = []
    for c in range(NCORES):
        b, h = c // 2, c % 2
        xt = np.ascontiguousarray(x[b].T)
        qrows = np.concatenate(
            [np.arange((2 * u + h) * 512, (2 * u + h + 1) * 512) for u in range(NU)]
        )
        xqt = np.ascontiguousarray(x[b][qrows].T)
        in_maps.append(
            dict(xt=xt, xqt=xqt, wq=wq, wk=wk, wv=wv,
                 wout=np.ascontiguousarray(w_out), bout=bout, mask=masks[h])
        )
    return in_maps


def _assemble(results):
    out = np.empty((B, N, DO), dtype=np.float32)
    for c in range(NCORES):
        b, h = c // 2, c % 2
        y = results[c]["y"]
        for u in range(NU):
            g = 2 * u + h
            out[b, g * 512:(g + 1) * 512, :] = y[u * 512:(u + 1) * 512, :]
    return out


def _run(inputs, **kw):
    nc = _get_nc()
    in_maps = _prep_inputs(
        np.asarray(inputs["x"], dtype=np.float32),
        np.asarray(inputs["w_qkv"], dtype=np.float32),
        np.asarray(inputs["w_out"], dtype=np.float32),
        np.asarray(inputs["b_out"], dtype=np.float32),
    )
    res = run_bass_kernel_spmd(nc, in_maps, list(range(NCORES)), **kw)
    return _assemble(res.results), res


def kernel(x, w_qkv, w_out, b_out):
    out, _ = _run(dict(x=x, w_qkv=w_qkv, w_out=w_out, b_out=b_out))
    return out



# revision 5
# speedup vs baseline: 1.1000x; 1.1000x over previous
"""Causal single-head attention (B=4, N=4096, d_in=1024, d_inner=512, d_out=1024)
for 8 Trainium2 NeuronCores.

Sharding: core c -> (batch b=c//2, half h=c%2). Core h of a pair owns the 4
global 512-row blocks {2t+h : t=0..3} of batch b, which serve BOTH as its
query blocks and as the key strips whose K/V projection it computes. The
missing (peer-parity) K/V strips arrive via 4 pipelined pair-wise AllGather
collectives (~1MB contribution each) that overlap with projection compute,
eliminating the duplicated K/V projection of the all-local scheme.

Key-slot layout after the gather is rank-major: kt slot s = r*4 + t holds
global strip 2t+r. Attention per query super-block u processes slots
{0..u} u {4..4+u}; causal masking enters only through a per-core 0/1 mask
input so the instruction stream stays SPMD-uniform. The rank1 diagonal
strip's blocks are truncated via free-dim offsets (queries below the
diagonal are never computed, read, or accumulated).

All matmuls run in fp16 (full PE rate, ~1e-3 rel err overall); PSUM
accumulation is fp32. The softmax denominator is accumulated on the Pool
engine (P_acc += P per block) with a single ones-matmul per super-block,
keeping the PE free of the per-block reduction matmuls.
"""

import sys

if "/opt/trn_rl_repo" not in sys.path:
    sys.path.insert(0, "/opt/trn_rl_repo")

import numpy as np

import concourse.bacc as bacc
import concourse.mybir as mybir
import concourse.tile as tile
from concourse.bass_utils import run_bass_kernel_spmd

P = 128
B, N, DIN, DI, DO = 4, 4096, 1024, 512, 1024
NCORES = 8
NQ = N // 2          # query rows per core (2048)
NT = 4               # rounds / query super-blocks per core
SCALE = float(DI) ** -0.5

F32 = mybir.dt.float32
F32R = mybir.dt.float32r
FP16 = mybir.dt.float16
AF = mybir.ActivationFunctionType

GROUPS = [[0, 1], [2, 3], [4, 5], [6, 7]]

_COMPILED = None


def _build():
    nc = bacc.Bacc(None, target_bir_lowering=False)

    xt_d = nc.dram_tensor("xt", [DIN, NQ], FP16, kind="ExternalInput")
    wq_d = nc.dram_tensor("wq", [DIN, DI], FP16, kind="ExternalInput")
    wk_d = nc.dram_tensor("wk", [DIN, DI], FP16, kind="ExternalInput")
    wv_d = nc.dram_tensor("wv", [DIN, DI], FP16, kind="ExternalInput")
    wout_d = nc.dram_tensor("wout", [DI, DO], FP16, kind="ExternalInput")
    bout_d = nc.dram_tensor("bout", [P, DO], FP16, kind="ExternalInput")
    mask_d = nc.dram_tensor("mask", [P, 8, 512], FP16, kind="ExternalInput")
    y_d = nc.dram_tensor("y", [NQ, DO], F32, kind="ExternalOutput")

    with tile.TileContext(nc) as tc:
        with tc.tile_pool(name="persist", bufs=1) as pp, tc.tile_pool(
            name="dram", bufs=1, space="DRAM"
        ) as dram:
            kt = pp.tile([P, 4, 8, 512], FP16)     # K^T [dk-chunk, slot, j]
            vt = pp.tile([P, 32, DI], FP16)        # V [kbslot, dv]
            qts = [pp.tile([P, 4, 512], FP16, name=f"qt{u}") for u in range(NT)]
            wout = pp.tile([P, 4, DO], FP16)
            bout = pp.tile([P, DO], FP16)
            mask = pp.tile([P, 8, 512], FP16)
            ones = pp.tile([P, P], FP16)
            ones_r = pp.tile([P, P], F32R)
            pacc = pp.tile([P, 512], F32R, bufs=2)

            cc_ins = [dram.tile([P, 4096], FP16, name=f"ccin{t}") for t in range(NT)]
            cc_outs = [
                dram.tile([2, P, 4096], FP16, name=f"ccout{t}") for t in range(NT)
            ]
            warm_in = dram.tile([P, 4], FP16)
            warm_out = dram.tile([2, P, 4], FP16)

            # ---- Phase A: project own-strip K/V/Q; pair-gather K/V ----
            pa = tc.tile_pool(name="phaseA", bufs=1)
            wp = pa.__enter__()
            psA_cm = tc.tile_pool(name="psA", bufs=1, space="PSUM")
            psA = psA_cm.__enter__()

            wk = wp.tile([P, 8, DI], FP16)
            wv = wp.tile([P, 8, DI], FP16)
            wq = wp.tile([P, 8, DI], FP16)

            # warm up ncfw before the first real collective
            warm_s = wp.tile([P, 4], FP16)
            nc.vector.memset(warm_s[:], 0.0)
            nc.sync.dma_start(warm_in[:], warm_s[:])
            nc.gpsimd.collective_compute(
                "AllGather",
                mybir.AluOpType.bypass,
                replica_groups=GROUPS,
                ins=[warm_in.opt()],
                outs=[warm_out.opt()],
            )

            wk_src = wk_d.ap().rearrange("(a p) n -> p a n", p=P)
            wv_src = wv_d.ap().rearrange("(a p) n -> p a n", p=P)
            for c in range(8):
                nc.gpsimd.dma_start(wk[:, c, :], wk_src[:, c, :])
            for c in range(8):
                nc.gpsimd.dma_start(wv[:, c, :], wv_src[:, c, :])
            nc.vector.memset(ones[:], 1.0)
            nc.vector.tensor_copy(ones_r[:], ones[:])

            late_dmas = {
                0: lambda: nc.sync.dma_start(
                    wq[:], wq_d.ap().rearrange("(a p) n -> p a n", p=P)
                ),
                1: lambda: (
                    nc.sync.dma_start(mask[:], mask_d.ap()),
                    nc.sync.dma_start(
                        wout[:], wout_d.ap().rearrange("(a p) n -> p a n", p=P)
                    ),
                ),
                2: lambda: nc.sync.dma_start(bout[:], bout_d.ap()),
            }

            for t in range(NT):
                xs = wp.tile([P, 8, 512], FP16, name=f"xs{t}", tag="xs", bufs=2)
                xs_src = xt_d.ap()[:, t * 512 : (t + 1) * 512].rearrange(
                    "(a p) j -> p a j", p=P
                )
                nc.sync.dma_start(xs[:, 0:4, :], xs_src[:, 0:4, :])
                nc.gpsimd.dma_start(xs[:, 4:8, :], xs_src[:, 4:8, :])
                if t in late_dmas:
                    late_dmas[t]()

                kstage = wp.tile([P, 4, 512], FP16, name=f"ks{t}", tag="ks", bufs=2)
                vstage = wp.tile([P, 4, 512], FP16, name=f"vs{t}", tag="vs", bufs=2)
                for dk in range(4):
                    kps = psA.tile([P, 512], F32, name=f"k{t}_{dk}", tag="kv", bufs=8)
                    for c in range(8):
                        nc.tensor.matmul(
                            kps[:], wk[:, c, dk * P : (dk + 1) * P], xs[:, c, :],
                            start=(c == 0), stop=(c == 7),
                        )
                    nc.vector.tensor_copy(kstage[:, dk, :], kps[:])
                for jsub in range(4):
                    vps = psA.tile([P, 512], F32, name=f"v{t}_{jsub}", tag="kv", bufs=8)
                    for c in range(8):
                        nc.tensor.matmul(
                            vps[:], xs[:, c, jsub * P : (jsub + 1) * P], wv[:, c, :],
                            start=(c == 0), stop=(c == 7),
                        )
                    nc.vector.tensor_copy(vstage[:, jsub, :], vps[:])

                nc.sync.dma_start(cc_ins[t][:, 0:2048], kstage[:])
                nc.sync.dma_start(cc_ins[t][:, 2048:4096], vstage[:])
                nc.gpsimd.collective_compute(
                    "AllGather",
                    mybir.AluOpType.bypass,
                    replica_groups=GROUPS,
                    ins=[cc_ins[t].opt()],
                    outs=[cc_outs[t].opt()],
                )
                for dq in range(4):
                    qps = psA.tile([P, 512], F32, name=f"q{t}_{dq}", tag="kv", bufs=8)
                    for c in range(8):
                        nc.tensor.matmul(
                            qps[:], wq[:, c, dq * P : (dq + 1) * P], xs[:, c, :],
                            start=(c == 0), stop=(c == 7),
                        )
                    nc.vector.tensor_copy(qts[t][:, dq, :], qps[:])

            # read gathered K/V back into rank-major slots. Emitted after all
            # phase-A loads so a pending collective never blocks them; round t
            # is consumed by attention super-block u=t, long after it lands.
            for t in range(NT):
                for r in range(2):
                    s = r * 4 + t
                    nc.sync.dma_start(
                        kt[:, :, s, :],
                        cc_outs[t][r][:, 0:2048].rearrange("p (a j) -> p a j", a=4),
                    )
                    nc.sync.dma_start(
                        vt[:, 4 * s : 4 * s + 4, :],
                        cc_outs[t][r][:, 2048:4096].rearrange("p (a j) -> p a j", a=4),
                    )

            psA_cm.__exit__(None, None, None)
            pa.__exit__(None, None, None)

            # ---- Phase B: attention per query super-block u ----
            pb = tc.tile_pool(name="phaseB", bufs=1)
            wp = pb.__enter__()
            psB_cm = tc.tile_pool(name="psB", bufs=1, space="PSUM")
            psB = psB_cm.__enter__()

            for u in range(NT):
                qt = qts[u]
                # (slot, kb, off, mask_row); first entry full-width, last
                # entry full-width (carries PSUM start/stop for the PV chain)
                order = []
                for s in range(u):
                    for kb in range(4):
                        order.append((s, kb, 0, None))
                for s in range(4, 4 + u):
                    for kb in range(4):
                        order.append((s, kb, 0, None))
                for kb in range(4):
                    order.append((4 + u, kb, kb * P, 4 + kb))
                for kb in range(4):
                    order.append((u, kb, 0, kb))
                nkb = len(order)

                outT = [
                    psB.tile([P, 512], F32, name=f"o{u}_{d}", tag=f"outT{d}", bufs=1)
                    for d in range(4)
                ]
                p_ts = []

                def attn_v(idx):
                    s, kb, off, _ = order[idx]
                    pt = p_ts[idx]
                    for dvc in range(4):
                        nc.tensor.matmul(
                            outT[dvc][:, off:512],
                            vt[:, 4 * s + kb, dvc * P : (dvc + 1) * P],
                            pt[:, off:512],
                            start=(idx == 0), stop=(idx == nkb - 1),
                        )

                LAG = 2
                for idx, (s, kb, off, mrow) in enumerate(order):
                    s_ps = psB.tile(
                        [P, 512], F32, name=f"s{u}_{idx}", tag="ka", bufs=3
                    )
                    for dkc in range(4):
                        nc.tensor.matmul(
                            s_ps[:, off:512],
                            kt[:, dkc, s, kb * P : (kb + 1) * P],
                            qt[:, dkc, off:512],
                            start=(dkc == 0), stop=(dkc == 3),
                        )
                    pt = wp.tile([P, 512], FP16, name=f"p{u}_{idx}", tag="pt", bufs=4)
                    p_ts.append(pt)
                    if mrow is not None and off == 0:
                        # halves: DVE mask-mul of half 0 runs under the
                        # ScalarE exp of half 1
                        nc.scalar.activation(pt[:, 0:256], s_ps[:, 0:256],
                                             AF.Exp, scale=SCALE)
                        nc.scalar.activation(pt[:, 256:512], s_ps[:, 256:512],
                                             AF.Exp, scale=SCALE)
                        nc.vector.tensor_mul(pt[:, 0:256], pt[:, 0:256],
                                             mask[:, mrow, 0:256])
                        nc.vector.tensor_mul(pt[:, 256:512], pt[:, 256:512],
                                             mask[:, mrow, 256:512])
                    else:
                        nc.scalar.activation(pt[:, off:512], s_ps[:, off:512],
                                             AF.Exp, scale=SCALE)
                        if mrow is not None:
                            nc.vector.tensor_mul(pt[:, off:512], pt[:, off:512],
                                                 mask[:, mrow, off:512])
                    if idx == 0:
                        nc.gpsimd.tensor_copy(pacc[:], pt[:])
                    else:
                        nc.gpsimd.tensor_add(pacc[:, off:512], pacc[:, off:512],
                                             pt[:, off:512])
                    if idx >= LAG:
                        attn_v(idx - LAG)
                for idx in range(nkb - LAG, nkb):
                    attn_v(idx)

                l_ps = psB.tile([P, 512], F32, name=f"l{u}", tag="l", bufs=1)
                nc.tensor.matmul(l_ps[:], ones_r[:, 0:P], pacc[:],
                                 start=True, stop=True)
                recip = wp.tile([P, 512], F32, name=f"r{u}", tag="recip", bufs=2)
                nc.vector.reciprocal(recip[:], l_ps[:])
                attn = [
                    wp.tile([P, 512], FP16, name=f"a{u}_{d}", tag=f"attn{d}", bufs=2)
                    for d in range(4)
                ]
                for dvc in range(4):
                    nc.vector.tensor_mul(attn[dvc][:], outT[dvc][:], recip[:])

                for ic in range(4):
                    y_s = wp.tile([P, DO], F32, name=f"y{u}_{ic}", tag="ys", bufs=2)
                    for doc in range(2):
                        y_ps = psB.tile(
                            [P, 512], F32, name=f"yp{u}_{ic}_{doc}", tag="ka", bufs=3
                        )
                        nc.tensor.matmul(
                            y_ps[:], ones[0:1, :],
                            bout[0:1, doc * 512 : (doc + 1) * 512],
                            start=True, stop=False,
                        )
                        for dvc in range(4):
                            nc.tensor.matmul(
                                y_ps[:],
                                attn[dvc][:, ic * P : (ic + 1) * P],
                                wout[:, dvc, doc * 512 : (doc + 1) * 512],
                                start=False, stop=(dvc == 3),
                            )
                        nc.scalar.activation(
                            y_s[:, doc * 512 : (doc + 1) * 512], y_ps[:], AF.Copy
                        )
                    nc.sync.dma_start(
                        y_d.ap().rearrange("(a p) n -> p a n", p=P)[:, u * 4 + ic, :],
                        y_s[:],
                    )
            psB_cm.__exit__(None, None, None)
            pb.__exit__(None, None, None)

    nc.compile()
    return nc


def _get_nc():
    global _COMPILED
    if _COMPILED is None:
        _COMPILED = _build()
    return _COMPILED


def _make_mask(h: int) -> np.ndarray:
    # rows 0..3: rank0 diagonal strip (global strip 2u vs queries 2u+h)
    # rows 4..7: rank1 diagonal strip (global strip 2u+1 vs queries 2u+h)
    pj = np.arange(P)[:, None, None]
    kb = np.arange(4)[None, :, None]
    il = np.arange(512)[None, None, :]
    m0 = (kb * P + pj - h * 512 - il) <= 0
    m1 = ((1 - h) * 512 + kb * P + pj - il) <= 0
    return np.concatenate([m0, m1], axis=1).astype(np.float16)


def _prep_inputs(x, w_qkv, w_out, b_out):
    wq = np.ascontiguousarray(w_qkv[:, 0:DI]).astype(np.float16)
    wk = np.ascontiguousarray(w_qkv[:, DI : 2 * DI]).astype(np.float16)
    wv = np.ascontiguousarray(w_qkv[:, 2 * DI : 3 * DI]).astype(np.float16)
    wout = np.ascontiguousarray(w_out).astype(np.float16)
    bout = np.broadcast_to(b_out.astype(np.float16), (P, DO)).copy()
    masks = [_make_mask(h) for h in range(2)]
    in_maps = []
    for c in range(NCORES):
        b, h = c // 2, c % 2
        qrows = np.concatenate(
            [np.arange((2 * t + h) * 512, (2 * t + h + 1) * 512) for t in range(NT)]
        )
        xt = np.ascontiguousarray(x[b][qrows].T).astype(np.float16)
        in_maps.append(
            dict(xt=xt, wq=wq, wk=wk, wv=wv, wout=wout, bout=bout, mask=masks[h])
        )
    return in_maps


def _assemble(results):
    out = np.empty((B, N, DO), dtype=np.float32)
    for c in range(NCORES):
        b, h = c // 2, c % 2
        y = results[c]["y"]
        for t in range(NT):
            g = 2 * t + h
            out[b, g * 512 : (g + 1) * 512, :] = y[t * 512 : (t + 1) * 512, :]
    return out


def _run(inputs, **kw):
    nc = _get_nc()
    in_maps = _prep_inputs(
        np.asarray(inputs["x"], dtype=np.float32),
        np.asarray(inputs["w_qkv"], dtype=np.float32),
        np.asarray(inputs["w_out"], dtype=np.float32),
        np.asarray(inputs["b_out"], dtype=np.float32),
    )
    res = run_bass_kernel_spmd(nc, in_maps, list(range(NCORES)), **kw)
    return _assemble(res.results), res


def kernel(x, w_qkv, w_out, b_out):
    out, _ = _run(dict(x=x, w_qkv=w_qkv, w_out=w_out, b_out=b_out))
    return out
